# revision 1
# baseline (speedup 1.0000x reference)
"""Performer (FAVOR+) attention block on 8 Trainium2 NeuronCores.

Math (per batch b):
    kp  = exp(k @ w.T - |k|^2/2) / sqrt(m)        [T, m]
    qp  = exp(q @ w.T - |q|^2/2) / sqrt(m)        [T, m]
    D   = qp @ (kp.sum(axis=0))                   [T, 1]
    kptv = v.T @ kp                               [d, m]
    y   = (qp @ kptv.T) / (D + eps)               [T, d]
    out = y @ proj_w.T                            [T, d]

Folds: out = (qp @ C') / (D + eps) with C' = (proj_w @ kptv).T [m, d],
removing the [T,d]x[d,d] projection matmul.

Sharding: 8 cores = 4 batches x 2 token-halves. Each core computes the
k-side (kp, kptv partial, ksum partial) for ITS half of the keys, then a
pairwise AllReduce (cores 2b, 2b+1) sums kptv/ksum; the q-side and output
are computed for the core's own query half. Matmul operands are bf16
(cast in-flight by DMA), accumulation fp32 in PSUM.

Layouts (SBUF [partition, free]):
  kt/qt  [d128 x 8dt, Tc]     token chunk of k/q, transposed on host
  wtxT   [m128, T512] psum    via wT-stationary matmuls (LDW reuse x4)
  xdT    [1, Tc]              -0.5*|x|^2 row, via (-1/2)-column matmuls
  kp_sb  [T128, m] bf16       via PE transpose of exp(wtxT)
  qpT_sb [m128, mt*TQ] bf16   direct exp output (no transpose needed)
  kptvT  [m128, d] psum/sbuf  kp-stationary matmuls, rhs v N=256 (LDW x4)
  C_sb   [m128, dout]         kptv-stationary matmuls over pwT
  out    [T128, dout]         qpT-stationary matmuls over C_sb + D column
"""

import math

import numpy as np

import concourse.bass as bass
import concourse.mybir as mybir
import concourse.tile as tile
from concourse import bacc, bass_utils
from concourse.masks import make_identity

F32 = mybir.dt.float32
BF16 = mybir.dt.bfloat16
AF = mybir.ActivationFunctionType

N_CORES = 8
B, T, D_MODEL, M = 4, 4096, 1024, 512
TC = T // 2                       # tokens per core (keys AND queries)
DT = D_MODEL // 128               # 8 d tiles
MT = M // 128                     # 4 m tiles
RC = TC // 128                    # 16 token tiles per core
NCH = TC // 512                   # 4 512-token chunks per core
NEG_HALF_LOG_M = -0.5 * math.log(M)
EPS = 1e-8
CC_GROUPS = [[0, 1], [2, 3], [4, 5], [6, 7]]


def _build_program():
    nc = bacc.Bacc("TRN2", target_bir_lowering=False, debug=False,
                   num_devices=N_CORES)

    kT_d = nc.dram_tensor("kT", [D_MODEL, TC], F32, kind="ExternalInput")
    v_d = nc.dram_tensor("v", [TC, D_MODEL], F32, kind="ExternalInput")
    qT_d = nc.dram_tensor("qT", [D_MODEL, TC], F32, kind="ExternalInput")
    wT_d = nc.dram_tensor("wT", [D_MODEL, M], F32, kind="ExternalInput")
    pwT_d = nc.dram_tensor("pwT", [D_MODEL, D_MODEL], F32, kind="ExternalInput")
    out_d = nc.dram_tensor("out", [TC, D_MODEL], F32, kind="ExternalOutput")

    with tile.TileContext(nc) as tc:
        with (
            tc.tile_pool(name="res", bufs=1) as res,
            tc.tile_pool(name="xstream", bufs=12) as xstream,
            tc.tile_pool(name="sqstream", bufs=2) as sqstream,
            tc.tile_pool(name="vstream", bufs=7) as vstream,
            tc.tile_pool(name="small", bufs=8) as small,
            tc.tile_pool(name="outp", bufs=3) as outp,
            tc.tile_pool(name="dram", bufs=1, space="DRAM") as dram,
        ):
            # ---- resident SBUF tensors ----
            wT_b = res.tile([128, DT * M], BF16, tag="wT_b")
            pwT_b = res.tile([128, DT * D_MODEL], BF16, tag="pwT_b")
            kp_sb = res.tile([128, RC * M], BF16, tag="kp_sb")
            qpT_sb = res.tile([128, MT * TC], BF16, tag="qpT_sb")
            kptvT_sb = res.tile([128, MT * D_MODEL], BF16, tag="kptvT_sb")
            kptv_sb = res.tile([128, DT * M], BF16, tag="kptv_sb")
            C_sb = res.tile([128, MT * D_MODEL], BF16, tag="C_sb")
            ksum_sb = res.tile([128, MT], BF16, tag="ksum_sb")
            xdT_k = res.tile([1, TC], BF16, tag="xdT_k")
            xdT_q = res.tile([1, TC], BF16, tag="xdT_q")
            ident = res.tile([128, 128], BF16, tag="ident")
            ones_col = res.tile([128, 1], BF16, tag="ones_col")
            ones_row = res.tile([1, 128], BF16, tag="ones_row")
            neghalf_col = res.tile([128, 1], BF16, tag="neghalf_col")
            expbias = res.tile([128, 1], F32, tag="expbias")

            cc_in = dram.tile([128, MT * D_MODEL + MT], BF16, tag="cc_in")
            cc_out = dram.tile([128, MT * D_MODEL + MT], BF16, tag="cc_out")


            def load_xt(src_d, chunks=None):
                """DMA-cast a [d, TC] operand into 8 bf16 [128, TC] tiles.

                Column-chunked (c-major) so the first wtx matmuls only wait
                for the 8 leading [128, 512] chunks, not the full 8 MB."""
                xt = [xstream.tile([128, TC], BF16, tag="xt", name=f"xt{dt}")
                      for dt in range(DT)]
                load_xt_chunks(src_d, xt, chunks if chunks is not None
                               else range(NCH))
                return xt

            def load_xt_chunks(src_d, xt, chunks):
                for c in chunks:
                    for dt in range(DT):
                        nc.gpsimd.dma_start(
                            xt[dt][:, c * 512:(c + 1) * 512],
                            src_d[dt * 128:(dt + 1) * 128,
                                  c * 512:(c + 1) * 512])

            def xd_phase(xt, xdT_out, psum_xd):
                """xdT_out[0, t] = -0.5 * sum_d x[d, t]^2  (bf16 row).

                The d-tile reduction happens on the vector engine (squares +
                adds); the 128-partition reduction is one M=1 matmul with a
                (-1/2)-column stationary per 512-token chunk."""
                for c in range(NCH):
                    xd = psum_xd.tile([1, 512], F32, tag="xdp")
                    lvl = []
                    for dt in range(DT):
                        sq = sqstream.tile([128, 512], BF16, tag="sq",
                                           name=f"sq{dt}", bufs=8)
                        nc.vector.tensor_mul(
                            sq[:], xt[dt][:, c * 512:(c + 1) * 512],
                            xt[dt][:, c * 512:(c + 1) * 512])
                        lvl.append(sq)
                    depth = 0
                    while len(lvl) > 1:
                        nxt = []
                        for i in range(0, len(lvl), 2):
                            s = sqstream.tile([128, 512], BF16,
                                              tag=f"sqa{depth}",
                                              name=f"sqa{depth}_{i}",
                                              bufs=max(2, 4 >> depth))
                            nc.vector.tensor_add(s[:], lvl[i][:],
                                                 lvl[i + 1][:])
                            nxt.append(s)
                        lvl = nxt
                        depth += 1
                    nc.tensor.matmul(xd[:], neghalf_col[:], lvl[0][:],
                                     start=True, stop=True)
                    nc.scalar.activation(
                        xdT_out[0:1, c * 512:(c + 1) * 512], xd[:],
                        AF.Copy)

            def wtx_phase(xt, xdT, psum_wtx, emit):
                """wtxT = w @ x - xd per m-tile; emit(mt, c, psum).

                mt==0 iterates chunk-major (matches the c-major DMA landing
                order, trading LDW reuse for no DMA stall); later passes
                iterate dt-major to amortize each LDWEIGHTS over 4 matmuls."""
                for mt in range(MT):
                    ps = [psum_wtx.tile([128, 512], F32, tag="wtx",
                                        name=f"wtx{c}")
                          for c in range(NCH)]
                    order = ([(dt, c) for c in range(NCH) for dt in range(DT)]
                             if mt == 0 else
                             [(dt, c) for dt in range(DT) for c in range(NCH)])
                    for dt, c in order:
                        lhs = wT_b[:, dt * M + mt * 128: dt * M + (mt + 1) * 128]
                        nc.tensor.matmul(
                            ps[c][:], lhs,
                            xt[dt][:, c * 512:(c + 1) * 512],
                            start=(dt == 0), stop=False)
                    for c in range(NCH):
                        nc.tensor.matmul(
                            ps[c][:], ones_row[:],
                            xdT[0:1, c * 512:(c + 1) * 512],
                            start=False, stop=True)
                        emit(mt, c, ps[c])

            # ================= K side =================
            # consts first (they gate the first xd/wtx matmuls), then the
            # leading kt column-chunk, then wT, then the rest.
            nc.gpsimd.memset(ones_col[:], 1.0)
            nc.gpsimd.memset(ones_row[:], 1.0)
            nc.gpsimd.memset(neghalf_col[:], -0.5)
            nc.gpsimd.memset(expbias[:], NEG_HALF_LOG_M)
            kt = load_xt(kT_d, chunks=[0])
            for dt in range(DT):
                nc.gpsimd.dma_start(
                    wT_b[:, dt * M:(dt + 1) * M],
                    wT_d[dt * 128:(dt + 1) * 128, :])
            make_identity(nc, ident[:])
            load_xt_chunks(kT_d, kt, range(1, NCH))
            with tc.tile_pool(name="psum_xdk", bufs=4,
                              space=bass.MemorySpace.PSUM) as psum_xd:
                xd_phase(kt, xdT_k, psum_xd)

            with (
                tc.tile_pool(name="psum_wtxk", bufs=6,
                             space=bass.MemorySpace.PSUM) as psum_wtx,
                tc.tile_pool(name="psum_trk", bufs=2,
                             space=bass.MemorySpace.PSUM) as psum_tr,
            ):
                def emit_k(mt, c, ps):
                    kpT = small.tile([128, 512], BF16, tag="kpT")
                    nc.scalar.activation(kpT[:], ps[:], AF.Exp,
                                         bias=expbias[:])
                    for sub in range(4):
                        r = c * 4 + sub
                        tr = psum_tr.tile([128, 128], BF16, tag="trk")
                        nc.tensor.transpose(
                            tr[:], kpT[:, sub * 128:(sub + 1) * 128],
                            ident[:])
                        nc.vector.tensor_copy(
                            kp_sb[:, r * M + mt * 128: r * M + (mt + 1) * 128],
                            tr[:])
                wtx_phase(kt, xdT_k, psum_wtx, emit_k)

            with tc.tile_pool(name="psum_ks", bufs=1,
                              space=bass.MemorySpace.PSUM) as psum_ks:
                ks = psum_ks.tile([1, M], F32, tag="ks")
                for r in range(RC):
                    nc.tensor.matmul(ks[:], ones_col[:],
                                     kp_sb[:, r * M:(r + 1) * M],
                                     start=(r == 0), stop=(r == RC - 1))
                ksr = small.tile([1, M], F32, tag="ksr")
                nc.scalar.activation(ksr[:], ks[:], AF.Copy)
                # scatter [1, 512] -> [128, 4]: cc_in[p, mt] = ksum[mt*128+p]
                nc.gpsimd.dma_start(
                    cc_in[:, MT * D_MODEL: MT * D_MODEL + MT],
                    ksr[0:1, :].rearrange("a (mt p) -> p (mt a)", p=128))

            # ---- KPTV partial ----
            with (
                tc.tile_pool(name="psum_kptv", bufs=1,
                             space=bass.MemorySpace.PSUM) as psum_kptv,
            ):
                pk = [psum_kptv.tile([128, D_MODEL], F32, tag=f"pk{mt}",
                                     name=f"pk{mt}")
                      for mt in range(MT)]
                vts = [vstream.tile([128, D_MODEL], BF16, tag="vt",
                                    name=f"vt{r}")
                       for r in range(RC)]
                PF = 6
                for r in range(PF):
                    nc.gpsimd.dma_start(vts[r][:],
                                        v_d[r * 128:(r + 1) * 128, :])
                for r in range(RC):
                    if r + PF < RC:
                        nc.gpsimd.dma_start(
                            vts[r + PF][:],
                            v_d[(r + PF) * 128:(r + PF + 1) * 128, :])
                    vt = vts[r]
                    for mt in range(MT):
                        lhs = kp_sb[:, r * M + mt * 128: r * M + (mt + 1) * 128]
                        for qtr in range(4):
                            nc.tensor.matmul(
                                pk[mt][:, qtr * 256:(qtr + 1) * 256],
                                lhs, vt[:, qtr * 256:(qtr + 1) * 256],
                                start=(r == 0), stop=(r == RC - 1))
                for mt in range(MT):
                    st = outp.tile([128, D_MODEL], BF16, tag="ccst",
                                   name="ccst", bufs=2)
                    nc.scalar.activation(st[:], pk[mt][:], AF.Copy)
                    nc.sync.dma_start(
                        cc_in[:, mt * D_MODEL:(mt + 1) * D_MODEL], st[:])

            # q-side loads issued before the AllReduce occupies gpsimd
            qt = load_xt(qT_d)

            # ---- pairwise AllReduce of kptv^T + ksum ----
            nc.gpsimd.collective_compute(
                "AllReduce", mybir.AluOpType.add, replica_groups=CC_GROUPS,
                ins=[cc_in.opt()], outs=[cc_out.opt()])
            nc.sync.dma_start(kptvT_sb[:], cc_out[:, 0: MT * D_MODEL])
            nc.sync.dma_start(ksum_sb[:],
                              cc_out[:, MT * D_MODEL: MT * D_MODEL + MT])

            # ================= Q side (overlaps the AllReduce) ============
            with tc.tile_pool(name="psum_xdq", bufs=4,
                              space=bass.MemorySpace.PSUM) as psum_xd:
                xd_phase(qt, xdT_q, psum_xd)
            with tc.tile_pool(name="psum_wtxq", bufs=6,
                              space=bass.MemorySpace.PSUM) as psum_wtx:
                def emit_q(mt, c, ps):
                    nc.scalar.activation(
                        qpT_sb[:, mt * TC + c * 512: mt * TC + (c + 1) * 512],
                        ps[:], AF.Exp, bias=expbias[:])
                wtx_phase(qt, xdT_q, psum_wtx, emit_q)

            # ---- load pwT late (only needed for C) ----
            for dt in range(DT):
                nc.gpsimd.dma_start(
                    pwT_b[:, dt * D_MODEL:(dt + 1) * D_MODEL],
                    pwT_d[dt * 128:(dt + 1) * 128, :])

            # ---- transpose kptv^T -> kptv ----
            with tc.tile_pool(name="psum_tr2", bufs=4,
                              space=bass.MemorySpace.PSUM) as psum_tr2:
                for mt in range(MT):
                    for dt in range(DT):
                        tr = psum_tr2.tile([128, 128], BF16, tag="tr2")
                        nc.tensor.transpose(
                            tr[:],
                            kptvT_sb[:, mt * D_MODEL + dt * 128:
                                     mt * D_MODEL + (dt + 1) * 128],
                            ident[:])
                        nc.vector.tensor_copy(
                            kptv_sb[:, dt * M + mt * 128: dt * M + (mt + 1) * 128],
                            tr[:])

            # ---- C' = (proj_w @ kptv).T [m, dout] ----
            with tc.tile_pool(name="psum_C", bufs=2,
                              space=bass.MemorySpace.PSUM) as psum_C:
                for mt in range(MT):
                    pc = psum_C.tile([128, D_MODEL], F32, tag="pc")
                    for dt in range(DT):
                        lhs = kptv_sb[:, dt * M + mt * 128: dt * M + (mt + 1) * 128]
                        for qtr in range(4):
                            nc.tensor.matmul(
                                pc[:, qtr * 256:(qtr + 1) * 256],
                                lhs,
                                pwT_b[:, dt * D_MODEL + qtr * 256:
                                      dt * D_MODEL + (qtr + 1) * 256],
                                start=(dt == 0), stop=(dt == DT - 1))
                    nc.scalar.activation(
                        C_sb[:, mt * D_MODEL:(mt + 1) * D_MODEL],
                        pc[:], AF.Copy)

            # ---- OUT: out = (qp @ C') / (D + eps) ----
            with (
                tc.tile_pool(name="psum_o", bufs=2,
                             space=bass.MemorySpace.PSUM) as psum_o,
                tc.tile_pool(name="psum_D", bufs=2,
                             space=bass.MemorySpace.PSUM) as psum_D,
            ):
                for r in range(RC):
                    po = psum_o.tile([128, D_MODEL], F32, tag="po")
                    pD = psum_D.tile([128, 1], F32, tag="pD")
                    for mt in range(MT):
                        lhs = qpT_sb[:, mt * TC + r * 128: mt * TC + (r + 1) * 128]
                        for qtr in range(4):
                            nc.tensor.matmul(
                                po[:, qtr * 256:(qtr + 1) * 256], lhs,
                                C_sb[:, mt * D_MODEL + qtr * 256:
                                     mt * D_MODEL + (qtr + 1) * 256],
                                start=(mt == 0), stop=(mt == MT - 1))
                        nc.tensor.matmul(pD[:], lhs, ksum_sb[:, mt:mt + 1],
                                         start=(mt == 0), stop=(mt == MT - 1))
                    Dp = small.tile([128, 1], F32, tag="Dp")
                    recD = small.tile([128, 1], F32, tag="recD")
                    nc.scalar.activation(Dp[:], pD[:], AF.Copy, bias=EPS)
                    nc.vector.reciprocal(recD[:], Dp[:])
                    ot = outp.tile([128, D_MODEL], F32, tag="ot")
                    for half in range(2):
                        nc.vector.tensor_scalar_mul(
                            ot[:, half * 512:(half + 1) * 512],
                            po[:, half * 512:(half + 1) * 512], recD[:])
                    nc.sync.dma_start(out_d[r * 128:(r + 1) * 128, :], ot[:])

    nc.compile()
    return nc


_NC_CACHE = None


def _get_program():
    global _NC_CACHE
    if _NC_CACHE is None:
        _NC_CACHE = _build_program()
    return _NC_CACHE


def _make_in_maps(q, k, v, w, proj_w):
    wT = np.ascontiguousarray(w.T)
    pwT = np.ascontiguousarray(proj_w.T)
    in_maps = []
    for c in range(N_CORES):
        b, h = divmod(c, 2)
        sl = slice(h * TC, (h + 1) * TC)
        in_maps.append({
            "kT": np.ascontiguousarray(k[b, sl].T),
            "v": np.ascontiguousarray(v[b, sl]),
            "qT": np.ascontiguousarray(q[b, sl].T),
            "wT": wT,
            "pwT": pwT,
        })
    return in_maps


def run(q, k, v, w, proj_w, trace=False, tmpdir=None):
    nc = _get_program()
    in_maps = _make_in_maps(q, k, v, w, proj_w)
    res = bass_utils.run_bass_kernel_spmd(
        nc, in_maps, core_ids=list(range(N_CORES)), trace=trace,
        tmpdir=tmpdir)
    out = np.empty((B, T, D_MODEL), dtype=np.float32)
    for c in range(N_CORES):
        b, h = divmod(c, 2)
        out[b, h * TC:(h + 1) * TC] = res.results[c]["out"]
    return out, res


def kernel(q, k, v, w, proj_w):
    out, _ = run(np.asarray(q, dtype=np.float32),
                 np.asarray(k, dtype=np.float32),
                 np.asarray(v, dtype=np.float32),
                 np.asarray(w, dtype=np.float32),
                 np.asarray(proj_w, dtype=np.float32))
    return out



# revision 6
# speedup vs baseline: 1.0915x; 1.0915x over previous
"""Performer (FAVOR+) attention block on 8 Trainium2 NeuronCores.

Math (per batch b, with the 1/sqrt(m) factors dropped -- they cancel
between numerator and denominator; eps is rescaled to eps*m):
    kp  = exp(k @ w.T - |k|^2/2)                  [T, m]
    qp  = exp(q @ w.T - |q|^2/2)                  [T, m]
    D   = qp @ (kp.sum(axis=0))                   [T, 1]
    kptv = v.T @ kp                               [d, m]
    C'  = kptv.T @ proj_w.T                       [m, d]   (folds proj)
    out = (qp @ C') / (D + eps*m)                 [T, d]

Sharding: 8 cores = 4 batches x 2 token-halves. Each core computes the
k-side for ITS half of the keys, folds the output projection into
C' = kptv_partial.T @ proj_w.T BEFORE the collective, then a pairwise
AllReduce (cores 2b, 2b+1) sums C'/ksum. The q-side runs during the
collective (q work > CC latency, so the CC is fully hidden); the
post-collective tail is just the out matmuls.

DMA strategy: descriptor posting costs ~0.6us per dma_start on the
issuing engine, so inputs are pre-swizzled ON THE HOST into exact
SBUF-tile layouts ([128, N] with multi-KB contiguous rows) and loaded
with few, large DMAs spread across the tensor/sync/vector/gpsimd
queues so no single queue serializes posting. All HBM I/O is bf16.

Layouts (SBUF [partition, free]):
  kt_b/qt_b [d128, 8dt x Tc]  host-swizzled k/q (transposed)
  kp_sb  [T128, RC*m] bf16    token-major: kt-stationary matmuls emit
                              [t, m] tiles directly, exp bias = -|k|^2/2
  qpT_sb [m128, MT*TC] bf16   m-major via w-stationary matmuls
  kv_sb  [d128, 8dt*m] bf16   kptv partial, d-major DIRECTLY via
                              v-stationary matmuls (no PE transposes)
  C_sb   [m128, dout] bf16    from the AllReduce
  out    [T128, dout]         qpT-stationary matmuls + D column
"""

import math

import numpy as np
import ml_dtypes

import concourse.bass as bass
import concourse.mybir as mybir
import concourse.tile as tile
from concourse import bacc, bass_utils
from concourse.masks import make_identity

F32 = mybir.dt.float32
BF16 = mybir.dt.bfloat16
AF = mybir.ActivationFunctionType
BF16_NP = ml_dtypes.bfloat16

N_CORES = 8
B, T, D_MODEL, M = 4, 4096, 1024, 512
TC = T // 2                       # tokens per core (keys AND queries)
DT = D_MODEL // 128               # 8 d tiles
MT = M // 128                     # 4 m tiles
RC = TC // 128                    # 16 token tiles per core
NCH = TC // 512                   # 4 512-token chunks per core
EPS_M = 1e-8 * M                  # eps rescaled by the dropped 1/m factor
CC_GROUPS = [[0, 1], [2, 3], [4, 5], [6, 7]]
CC_COFF = 32                      # C' offset in the cc payload (aligned)
CC_COLS = CC_COFF + MT * D_MODEL
N_DUMMY = 10                      # HAM warm-keeper links (CC insurance)


def _build_program():
    nc = bacc.Bacc("TRN2", target_bir_lowering=False, debug=False,
                   num_devices=N_CORES)

    kT_d = nc.dram_tensor("kTsw", [128, DT * TC], BF16, kind="ExternalInput")
    qT_d = nc.dram_tensor("qTsw", [128, DT * TC], BF16, kind="ExternalInput")
    v_d = nc.dram_tensor("vsw", [128, RC * D_MODEL], BF16,
                         kind="ExternalInput")
    wT_d = nc.dram_tensor("wTsw", [128, DT * M], BF16, kind="ExternalInput")
    pwT_d = nc.dram_tensor("pwTsw", [128, DT * D_MODEL], BF16,
                           kind="ExternalInput")
    out_d = nc.dram_tensor("out", [TC, D_MODEL], BF16, kind="ExternalOutput")

    with tile.TileContext(nc) as tc:
        with (
            tc.tile_pool(name="res", bufs=1) as res,
            tc.tile_pool(name="sqstream", bufs=2) as sqstream,
            tc.tile_pool(name="small", bufs=8) as small,
            tc.tile_pool(name="outp", bufs=3) as outp,
            tc.tile_pool(name="dram", bufs=1, space="DRAM") as dram,
        ):
            # ---- resident SBUF tensors ----
            kt_b = res.tile([128, DT * TC], BF16, tag="kt_b")
            qt_b = res.tile([128, DT * TC], BF16, tag="qt_b")
            vt_b = res.tile([128, RC * D_MODEL], BF16, tag="vt_b")
            wT_b = res.tile([128, DT * M], BF16, tag="wT_b")
            pwT_b = res.tile([128, DT * D_MODEL], BF16, tag="pwT_b")
            kp_sb = res.tile([128, RC * M], BF16, tag="kp_sb")
            qpT_sb = res.tile([128, MT * TC], BF16, tag="qpT_sb")
            kv_sb = res.tile([128, DT * M], BF16, tag="kv_sb")
            C_sb = res.tile([128, MT * D_MODEL], BF16, tag="C_sb")
            ksum_sb = res.tile([128, MT], BF16, tag="ksum_sb")
            xdc_k = res.tile([128, RC], F32, tag="xdc_k")
            xdT_q = res.tile([1, TC], BF16, tag="xdT_q")
            ident = res.tile([128, 128], BF16, tag="ident")
            ones_col = res.tile([128, 1], BF16, tag="ones_col")
            ones_row = res.tile([1, 128], BF16, tag="ones_row")
            neghalf_col = res.tile([128, 1], BF16, tag="neghalf_col")
            junkA = res.tile([128, 1024], BF16, tag="junkA")
            junkB = res.tile([128, 1024], BF16, tag="junkB")

            cc_in = dram.tile([128, CC_COLS], BF16, tag="cc_in")
            cc_out = dram.tile([128, CC_COLS], BF16, tag="cc_out")

            # ---- loads. Only 3 DMA-capable queues (sync/gpsimd/scalar),
            # each ~150 GB/s, sharing ~358 GB/s of HBM. Phase 1 puts the
            # critical set (k chunk0 + wT) on ALL THREE queues so the
            # first matmuls start ~7us in; then each queue streams its
            # bulk assignment in need-order. ----
            nc.gpsimd.memset(ones_col[:], 1.0)
            nc.gpsimd.memset(neghalf_col[:], -0.5)
            nc.gpsimd.memset(ones_row[:], 1.0)
            # phase 1: k chunk0 (8 tiles) + wT (4 slices) across 3 queues
            for dt in range(3):
                nc.sync.dma_start(kt_b[:, dt * TC:dt * TC + 512],
                                  kT_d[:, dt * TC:dt * TC + 512])
            for dt in range(3, 6):
                nc.gpsimd.dma_start(kt_b[:, dt * TC:dt * TC + 512],
                                    kT_d[:, dt * TC:dt * TC + 512])
            for dt in range(6, 8):
                nc.scalar.dma_start(kt_b[:, dt * TC:dt * TC + 512],
                                    kT_d[:, dt * TC:dt * TC + 512])
            for i in range(2):
                nc.sync.dma_start(wT_b[:, i * 1024:(i + 1) * 1024],
                                  wT_d[:, i * 1024:(i + 1) * 1024])
            nc.gpsimd.dma_start(wT_b[:, 2048:3072], wT_d[:, 2048:3072])
            nc.scalar.dma_start(wT_b[:, 3072:4096], wT_d[:, 3072:4096])
            # phase 2: gpsimd: k chunk1 (fine granularity -- r4-7 unblock
            # early), then chunks 2-3, then pwT; scalar: v; sync: qT.
            for dt in range(DT):
                nc.gpsimd.dma_start(
                    kt_b[:, dt * TC + 512:dt * TC + 1024],
                    kT_d[:, dt * TC + 512:dt * TC + 1024])
            for i in range(8):
                nc.scalar.dma_start(vt_b[:, i * 2048:(i + 1) * 2048],
                                    v_d[:, i * 2048:(i + 1) * 2048])
            for i in range(4):
                nc.sync.dma_start(qt_b[:, i * 4096:(i + 1) * 4096],
                                  qT_d[:, i * 4096:(i + 1) * 4096])
            for dt in range(DT):
                nc.gpsimd.dma_start(kt_b[:, dt * TC + 1024:(dt + 1) * TC],
                                    kT_d[:, dt * TC + 1024:(dt + 1) * TC])
            for i in range(4):
                nc.gpsimd.dma_start(pwT_b[:, i * 2048:(i + 1) * 2048],
                                    pwT_d[:, i * 2048:(i + 1) * 2048])
            make_identity(nc, ident[:])
            nc.gpsimd.memset(junkA[:], 0.0)

            def sq_chunk(xt_b, c, tag):
                """s[p, t] = sum_dt x[dt*128+p, t]^2 for one 512-col chunk."""
                lvl = []
                for dt in range(DT):
                    sq = sqstream.tile([128, 512], BF16, tag="sq",
                                       name=f"sq{tag}{dt}", bufs=8)
                    nc.vector.tensor_mul(
                        sq[:], xt_b[:, dt * TC + c * 512:dt * TC + (c + 1) * 512],
                        xt_b[:, dt * TC + c * 512:dt * TC + (c + 1) * 512])
                    lvl.append(sq)
                depth = 0
                while len(lvl) > 1:
                    nxt = []
                    for i in range(0, len(lvl), 2):
                        s = sqstream.tile([128, 512], BF16,
                                          tag=f"sqa{depth}",
                                          name=f"sqa{tag}{depth}_{i}",
                                          bufs=max(2, 4 >> depth))
                        nc.vector.tensor_add(s[:], lvl[i][:], lvl[i + 1][:])
                        nxt.append(s)
                    lvl = nxt
                    depth += 1
                return lvl[0]

            # ================= K side (token-major) =================
            with (
                tc.tile_pool(name="psum_wtxk", bufs=5,
                             space=bass.MemorySpace.PSUM) as psum_wtx,
                tc.tile_pool(name="psum_xdk", bufs=2,
                             space=bass.MemorySpace.PSUM) as psum_xd,
            ):
                for c in range(NCH):
                    s_c = sq_chunk(kt_b, c, "k")
                    for rl in range(4):
                        r = c * 4 + rl
                        xdp = psum_xd.tile([128, 1], F32, tag="xdp")
                        nc.tensor.matmul(xdp[:],
                                         s_c[:, rl * 128:(rl + 1) * 128],
                                         neghalf_col[:], start=True, stop=True)
                        nc.scalar.activation(xdc_k[:, r:r + 1], xdp[:],
                                             AF.Copy)
                        ps = psum_wtx.tile([128, M], F32, tag="wtx")
                        for dt in range(DT):
                            nc.tensor.matmul(
                                ps[:],
                                kt_b[:, dt * TC + r * 128:dt * TC + (r + 1) * 128],
                                wT_b[:, dt * M:(dt + 1) * M],
                                start=(dt == 0), stop=(dt == DT - 1))
                        nc.scalar.activation(
                            kp_sb[:, r * M:(r + 1) * M], ps[:], AF.Exp,
                            bias=xdc_k[:, r:r + 1])

            # ---- kptv partial d-major (v-stationary) + ksum partial ----
            with tc.tile_pool(name="psum_ks", bufs=1,
                              space=bass.MemorySpace.PSUM) as psum_ks:
                ks = psum_ks.tile([128, MT], F32, tag="ks")
                for wave in range(2):
                    with tc.tile_pool(name=f"psum_kptv{wave}", bufs=1,
                                      space=bass.MemorySpace.PSUM) as psum_kptv:
                        pk = {dt: psum_kptv.tile([128, M], F32,
                                                 tag=f"pk{dt}", name=f"pk{dt}")
                              for dt in range(4 * wave, 4 * wave + 4)}
                        for r in range(RC):
                            for dt in pk:
                                nc.tensor.matmul(
                                    pk[dt][:],
                                    vt_b[:, r * D_MODEL + dt * 128:
                                         r * D_MODEL + (dt + 1) * 128],
                                    kp_sb[:, r * M:(r + 1) * M],
                                    start=(r == 0), stop=(r == RC - 1))
                            if wave == 0:
                                for mt in range(MT):
                                    nc.tensor.matmul(
                                        ks[:, mt:mt + 1],
                                        kp_sb[:, r * M + mt * 128:
                                              r * M + (mt + 1) * 128],
                                        ones_col[:], start=(r == 0),
                                        stop=(r == RC - 1))
                        for dt in pk:
                            nc.scalar.activation(
                                kv_sb[:, dt * M:(dt + 1) * M], pk[dt][:],
                                AF.Copy)
                ks_st = small.tile([128, MT], BF16, tag="ks_st")
                nc.scalar.activation(ks_st[:], ks[:], AF.Copy)
                nc.sync.dma_start(cc_in[:, 0:MT], ks_st[:])

            # ---- C' partial = kptv_partial^T @ proj_w^T  [m, dout] ----
            with tc.tile_pool(name="psum_C", bufs=2,
                              space=bass.MemorySpace.PSUM) as psum_C:
                for mt in range(MT):
                    pc = psum_C.tile([128, D_MODEL], F32, tag="pc")
                    for dt in range(DT):
                        lhs = kv_sb[:, dt * M + mt * 128:
                                    dt * M + (mt + 1) * 128]
                        for h in range(2):
                            nc.tensor.matmul(
                                pc[:, h * 512:(h + 1) * 512], lhs,
                                pwT_b[:, dt * D_MODEL + h * 512:
                                      dt * D_MODEL + (h + 1) * 512],
                                start=(dt == 0), stop=(dt == DT - 1))
                    st = outp.tile([128, D_MODEL], BF16, tag="ccst",
                                   name="ccst", bufs=2)
                    nc.scalar.activation(st[:], pc[:], AF.Copy)
                    nc.sync.dma_start(
                        cc_in[:, CC_COFF + mt * D_MODEL:
                              CC_COFF + (mt + 1) * D_MODEL], st[:])

            # ---- pairwise AllReduce of C' + ksum ----
            nc.gpsimd.collective_compute(
                "AllReduce", mybir.AluOpType.add, replica_groups=CC_GROUPS,
                ins=[cc_in.opt()], outs=[cc_out.opt()])
            nc.sync.dma_start(ksum_sb[:], cc_out[:, 0:MT])
            nc.sync.dma_start(C_sb[:], cc_out[:, CC_COFF:CC_COFF + MT * D_MODEL])

            # ================= Q side (hides the AllReduce) ============
            with (
                tc.tile_pool(name="psum_wtxq", bufs=4,
                             space=bass.MemorySpace.PSUM) as psum_wtx,
                tc.tile_pool(name="psum_xdq", bufs=2,
                             space=bass.MemorySpace.PSUM) as psum_xd,
            ):
                for c in range(NCH):
                    s_c = sq_chunk(qt_b, c, "q")
                    xdp = psum_xd.tile([1, 512], F32, tag="xdq")
                    nc.tensor.matmul(xdp[:], neghalf_col[:], s_c[:],
                                     start=True, stop=True)
                    nc.scalar.activation(xdT_q[0:1, c * 512:(c + 1) * 512],
                                         xdp[:], AF.Copy)
                for mt in range(MT):
                    for c in range(NCH):
                        wq = psum_wtx.tile([128, 512], F32, tag="wq")
                        for dt in range(DT):
                            nc.tensor.matmul(
                                wq[:],
                                wT_b[:, dt * M + mt * 128:
                                     dt * M + (mt + 1) * 128],
                                qt_b[:, dt * TC + c * 512:dt * TC + (c + 1) * 512],
                                start=(dt == 0), stop=False)
                        nc.tensor.matmul(wq[:], ones_row[:],
                                         xdT_q[0:1, c * 512:(c + 1) * 512],
                                         start=False, stop=True)
                        nc.scalar.activation(
                            qpT_sb[:, mt * TC + c * 512:
                                   mt * TC + (c + 1) * 512],
                            wq[:], AF.Exp)

            # ---- HAM warm-keeper: paced dummy matmuls (CC insurance) ----
            with tc.tile_pool(name="psum_dummy", bufs=2,
                              space=bass.MemorySpace.PSUM) as psum_dummy:
                for i in range(N_DUMMY):
                    src, dst = (junkA, junkB) if i % 2 == 0 else (junkB, junkA)
                    nc.vector.tensor_copy(dst[:], src[:])
                    dp = psum_dummy.tile([128, 16], F32, tag="dp")
                    nc.tensor.matmul(dp[:], ident[:, 0:128],
                                     dst[:, 0:16], start=True, stop=True)

            # ---- OUT: out = (qp @ C') / (D + eps*m) ----
            with (
                tc.tile_pool(name="psum_o", bufs=2,
                             space=bass.MemorySpace.PSUM) as psum_o,
                tc.tile_pool(name="psum_D", bufs=2,
                             space=bass.MemorySpace.PSUM) as psum_D,
            ):
                for r in range(RC):
                    po = psum_o.tile([128, D_MODEL], F32, tag="po")
                    pD = psum_D.tile([128, 1], F32, tag="pD")
                    for mt in range(MT):
                        lhs = qpT_sb[:, mt * TC + r * 128:
                                     mt * TC + (r + 1) * 128]
                        for h in range(2):
                            nc.tensor.matmul(
                                po[:, h * 512:(h + 1) * 512], lhs,
                                C_sb[:, mt * D_MODEL + h * 512:
                                     mt * D_MODEL + (h + 1) * 512],
                                start=(mt == 0), stop=(mt == MT - 1))
                        nc.tensor.matmul(pD[:], lhs, ksum_sb[:, mt:mt + 1],
                                         start=(mt == 0), stop=(mt == MT - 1))
                    Dp = small.tile([128, 1], F32, tag="Dp")
                    recD = small.tile([128, 1], F32, tag="recD")
                    nc.scalar.activation(Dp[:], pD[:], AF.Copy, bias=EPS_M)
                    nc.vector.reciprocal(recD[:], Dp[:])
                    ot = outp.tile([128, D_MODEL], BF16, tag="ot")
                    for h in range(2):
                        nc.vector.tensor_scalar_mul(
                            ot[:, h * 512:(h + 1) * 512],
                            po[:, h * 512:(h + 1) * 512], recD[:])
                    nc.sync.dma_start(out_d[r * 128:(r + 1) * 128, :], ot[:])

    nc.compile()
    return nc


_NC_CACHE = None


def _get_program():
    global _NC_CACHE
    if _NC_CACHE is None:
        _NC_CACHE = _build_program()
    return _NC_CACHE


def _swz(xT):
    """[D, T] -> [128, DT*T] with sw[p, dt*T + t] = xT[dt*128+p, t]."""
    d, t = xT.shape
    return np.ascontiguousarray(
        xT.reshape(d // 128, 128, t).transpose(1, 0, 2).reshape(128, -1))


def _make_in_maps(q, k, v, w, proj_w):
    wsw = _swz(w.T.astype(BF16_NP))
    pwsw = _swz(proj_w.T.astype(BF16_NP))
    in_maps = []
    for c in range(N_CORES):
        b, h = divmod(c, 2)
        sl = slice(h * TC, (h + 1) * TC)
        in_maps.append({
            "kTsw": _swz(k[b, sl].astype(BF16_NP).T),
            "qTsw": _swz(q[b, sl].astype(BF16_NP).T),
            "vsw": _swz(v[b, sl].astype(BF16_NP)),
            "wTsw": wsw,
            "pwTsw": pwsw,
        })
    return in_maps


def run(q, k, v, w, proj_w, trace=False, tmpdir=None):
    nc = _get_program()
    in_maps = _make_in_maps(q, k, v, w, proj_w)
    res = bass_utils.run_bass_kernel_spmd(
        nc, in_maps, core_ids=list(range(N_CORES)), trace=trace,
        tmpdir=tmpdir)
    out = np.empty((B, T, D_MODEL), dtype=np.float32)
    for c in range(N_CORES):
        b, h = divmod(c, 2)
        out[b, h * TC:(h + 1) * TC] = res.results[c]["out"].astype(np.float32)
    return out, res


def kernel(q, k, v, w, proj_w):
    out, _ = run(np.asarray(q, dtype=np.float32),
                 np.asarray(k, dtype=np.float32),
                 np.asarray(v, dtype=np.float32),
                 np.asarray(w, dtype=np.float32),
                 np.asarray(proj_w, dtype=np.float32))
    return out


# revision 14
# speedup vs baseline: 1.4914x; 1.3664x over previous
"""Performer (FAVOR+) attention block on 8 Trainium2 NeuronCores.

Math (per batch b; the 1/sqrt(m) normalizations cancel between
numerator and denominator and a 64x scale is folded into the exp so
fp8 values stay in the normal range; eps is rescaled accordingly):
    kp' = 64*exp(k @ w.T - |k|^2/2)               [T, m]
    qp' = 64*exp(q @ w.T - |q|^2/2)               [T, m]
    ksum = kp'.sum(axis=0)/64                     [m]
    kptv'' = v.T @ kp'                            [d, m]
    C''  = kptv''.T @ proj_w.T                    [m, d]
    out  = (qp' @ C'') / (64*(qp' @ ksum) + 4096*m*eps)

Sharding: 8 cores = 4 batches x 2 token-halves; pairwise AllReduce of
C''+ksum (cores 2b, 2b+1); the q-side hides the collective; the tail
is just the out matmuls.

All matmul operands are fp8e4 with perf_mode=DoubleRow (K=256 per
matmul, ~1.7x bf16 throughput); accumulation is fp32 in PSUM. Inputs
are cast to fp8 and PAIR-INTERLEAVED on the host so every DoubleRow
operand is a contiguous [128, 2, N] access pattern (middle dim = the
two 128-row contraction planes). HBM I/O is fp8 in / bf16 out.

DMA: only 3 issue queues (sync/gpsimd/scalar), ~150 GB/s each sharing
~358 GB/s HBM; the critical set (k r0-3 + w) goes first across all 3.
"""

import math

import numpy as np
import ml_dtypes

import concourse.bass as bass
import concourse.mybir as mybir
import concourse.tile as tile
from concourse import bacc, bass_utils

F32 = mybir.dt.float32
BF16 = mybir.dt.bfloat16
FP8 = mybir.dt.float8e4
AF = mybir.ActivationFunctionType
DR = mybir.MatmulPerfMode.DoubleRow
BF16_NP = ml_dtypes.bfloat16
FP8_NP = ml_dtypes.float8_e4m3

N_CORES = 8
B, T, D_MODEL, M = 4, 4096, 1024, 512
TC = T // 2                       # tokens per core (keys AND queries)
DT = D_MODEL // 128               # 8 d tiles
MT = M // 128                     # 4 m tiles
RC = TC // 128                    # 16 token tiles per core
NCH = TC // 512                   # 4 512-token chunks per core
EPS_M = 1e-8 * M
LOG64 = math.log(64.0)            # folded into the exp bias
KV_SCALE = 1.0 / 128.0            # kv8 = kptv_u/2 (fp8 range)
KS_SCALE = 1.0 / 4096.0           # ksum8 = ksum_u/64 (fp8 range)
DIV_SCALE = 32.0                  # out = po / (32*pD + 32*m*eps)
DIV_BIAS = 32.0 * EPS_M
CC_GROUPS = [[0, 1], [2, 3], [4, 5], [6, 7]]
CC_COFF = 512
CC_COLS = CC_COFF + MT * D_MODEL
N_DUMMY = 8


def _pair(ap):
    """View a [128, 2*N] slice as the DoubleRow [128, 2, N] operand."""
    return ap.rearrange("p (o n) -> p o n", o=2)


def _build_program():
    nc = bacc.Bacc("TRN2", target_bir_lowering=False, debug=False,
                   num_devices=N_CORES)

    k_d = nc.dram_tensor("k8", [128, RC * 1024], FP8, kind="ExternalInput")
    q_d = nc.dram_tensor("q8", [128, DT * 2048], FP8, kind="ExternalInput")
    v_d = nc.dram_tensor("v8", [128, RC * 1024], FP8, kind="ExternalInput")
    wk_d = nc.dram_tensor("wk8", [128, DT * M], FP8, kind="ExternalInput")
    wq_d = nc.dram_tensor("wq8", [128, DT * M], FP8, kind="ExternalInput")
    pw_d = nc.dram_tensor("pw8", [128, DT * D_MODEL], FP8,
                          kind="ExternalInput")
    id_d = nc.dram_tensor("ident", [128, 128], BF16, kind="ExternalInput")
    out_d = nc.dram_tensor("out", [TC, D_MODEL], BF16, kind="ExternalOutput")

    with tile.TileContext(nc) as tc:
        with (
            tc.tile_pool(name="res", bufs=1) as res,
            tc.tile_pool(name="sqstream", bufs=2) as sqstream,
            tc.tile_pool(name="small", bufs=8) as small,
            tc.tile_pool(name="outp", bufs=3) as outp,
            tc.tile_pool(name="dram", bufs=1, space="DRAM") as dram,
        ):
            # ---- resident SBUF tensors (matmul operands fp8) ----
            # kt8[p, r*1024 + j*256 + o*128 + t'] = k[(2j+o)*128+p, r*128+t']
            kt8 = res.tile([128, RC * 1024], FP8, tag="kt8")
            # qt8[p, j*4096 + c*1024 + o*512 + t'] = qT[(2j+o)*128+p, c*512+t']
            qt8 = res.tile([128, DT * 2048], FP8, tag="qt8")
            # vt8[p, rr*2048 + dt*256 + o*128 + d'] = v[(2rr+o)*128+p, dt*128+d']
            vt8 = res.tile([128, RC * 1024], FP8, tag="vt8")
            # wk8[p, j*1024 + o*512 + m] = wT[(2j+o)*128+p, m]
            wk8 = res.tile([128, DT * M], FP8, tag="wk8")
            # wq8[p, j*1024 + mt*256 + o*128 + m'] = wT[(2j+o)*128+p, mt*128+m']
            wq8 = res.tile([128, DT * M], FP8, tag="wq8")
            # pw8[p, j*2048 + h*1024 + o*512 + n'] = pwT[(2j+o)*128+p, h*512+n']
            pw8 = res.tile([128, DT * D_MODEL], FP8, tag="pw8")
            # kp8[p, r*512 + m] = kp'[r*128+p, m]
            kp8 = res.tile([128, RC * M], FP8, tag="kp8")
            # qp8[p, j*4096 + r*256 + o*128 + t'] = qp'T[(2j+o)*128+p, r*128+t']
            qp8 = res.tile([128, MT * TC], FP8, tag="qp8")
            # kv8[p, j*1024 + mt*256 + o*128 + m'] = kptv''[(2j+o)*128+p, mt*128+m']
            kv8 = res.tile([128, DT * M], FP8, tag="kv8")
            # C8[p, j*2048 + h*1024 + o*512 + n'] = C''[(2j+o)*128+p, h*512+n']
            C8 = res.tile([128, MT * D_MODEL], FP8, tag="C8")
            ksum8 = res.tile([128, MT], FP8, tag="ksum8")
            xdc_k = res.tile([128, RC], F32, tag="xdc_k")
            xdT_q = res.tile([1, TC], BF16, tag="xdT_q")
            ident = res.tile([128, 128], BF16, tag="ident")
            ones_pair8 = res.tile([128, 32], FP8, tag="ones_pair8")
            ones_row = res.tile([1, 128], BF16, tag="ones_row")
            neghalf_col = res.tile([128, 1], BF16, tag="neghalf_col")
            junkA = res.tile([128, 1024], BF16, tag="junkA")
            junkB = res.tile([128, 1024], BF16, tag="junkB")

            cc_in = dram.tile([128, CC_COLS], FP8, tag="cc_in")
            cc_out = dram.tile([128, CC_COLS], FP8, tag="cc_out")

            # ---- loads: critical set (k r0-3, w) across all 3 queues ----
            nc.gpsimd.memset(ones_pair8[:], 1.0)
            nc.gpsimd.memset(neghalf_col[:], -0.5)
            nc.gpsimd.memset(ones_row[:], 1.0)
            for r in (0, 1):
                nc.sync.dma_start(kt8[:, r * 1024:(r + 1) * 1024],
                                  k_d[:, r * 1024:(r + 1) * 1024])
            for r in (2, 3):
                nc.gpsimd.dma_start(kt8[:, r * 1024:(r + 1) * 1024],
                                    k_d[:, r * 1024:(r + 1) * 1024])
            nc.scalar.dma_start(wk8[:], wk_d[:, :])
            nc.scalar.dma_start(ident[:], id_d[:, :])
            # warm the exp table early (off the critical path)
            wexp = small.tile([128, 1], BF16, tag="wexp")
            nc.scalar.activation(wexp[:], neghalf_col[:], AF.Exp)
            # bulk: k r4-15 split across all 3 queues, then v/q/w/pw
            nc.sync.dma_start(kt8[:, 4096:6144], k_d[:, 4096:6144])
            nc.gpsimd.dma_start(kt8[:, 6144:8192], k_d[:, 6144:8192])
            nc.scalar.dma_start(kt8[:, 8192:10240], k_d[:, 8192:10240])
            nc.sync.dma_start(kt8[:, 10240:12288], k_d[:, 10240:12288])
            nc.gpsimd.dma_start(kt8[:, 12288:14336], k_d[:, 12288:14336])
            nc.scalar.dma_start(kt8[:, 14336:16384], k_d[:, 14336:16384])
            for i in range(4):
                nc.scalar.dma_start(vt8[:, i * 4096:(i + 1) * 4096],
                                    v_d[:, i * 4096:(i + 1) * 4096])
            nc.gpsimd.dma_start(wq8[:], wq_d[:, :])
            for i in range(2):
                nc.sync.dma_start(qt8[:, i * 8192:(i + 1) * 8192],
                                  q_d[:, i * 8192:(i + 1) * 8192])
            for i in range(2):
                nc.gpsimd.dma_start(pw8[:, i * 4096:(i + 1) * 4096],
                                    pw_d[:, i * 4096:(i + 1) * 4096])
            nc.gpsimd.memset(junkA[:], 0.0)

            # ================= K side (token-major, DoubleRow) ==========
            with (
                tc.tile_pool(name="psum_wtxk", bufs=5,
                             space=bass.MemorySpace.PSUM) as psum_wtx,
                tc.tile_pool(name="psum_gram", bufs=2,
                             space=bass.MemorySpace.PSUM) as psum_gram,
                tc.tile_pool(name="psum_ks", bufs=1,
                             space=bass.MemorySpace.PSUM) as psum_ks,
            ):
                ks = psum_ks.tile([16, M], F32, tag="ks")
                for r in range(RC):
                    # -|k|^2/2 via the k-Gram diagonal, on the PE: the 4
                    # gram matmuls reuse the same stationary kt pairs as
                    # the wtx matmuls below; the diagonal is extracted by
                    # an identity mask (DVE) + activation accum_out with
                    # scale=-1/2 and bias=log(64)/128 (summed 128x).
                    gram = psum_gram.tile([128, 128], F32, tag="gram")
                    for j in range(4):
                        kpair = _pair(kt8[:, r * 1024 + j * 256:
                                          r * 1024 + (j + 1) * 256])
                        nc.tensor.matmul(gram[:], kpair, kpair,
                                         start=(j == 0), stop=(j == 3),
                                         perf_mode=DR)
                    dv = sqstream.tile([128, 128], BF16, tag="dv", bufs=3)
                    nc.vector.tensor_mul(dv[:], gram[:], ident[:])
                    scr = sqstream.tile([128, 128], BF16, tag="scr", bufs=2)
                    nc.scalar.activation(scr[:], dv[:], AF.Copy, scale=-0.5,
                                         bias=LOG64 / 128.0,
                                         accum_out=xdc_k[:, r:r + 1])
                    # wtx[t, m] over 4 dt-pairs, DoubleRow
                    ps = psum_wtx.tile([128, M], F32, tag="wtx")
                    for j in range(4):
                        nc.tensor.matmul(
                            ps[:],
                            _pair(kt8[:, r * 1024 + j * 256:
                                      r * 1024 + (j + 1) * 256]),
                            _pair(wk8[:, j * 1024:(j + 1) * 1024]),
                            start=(j == 0), stop=(j == 3), perf_mode=DR)
                    nc.scalar.activation(kp8[:, r * M:(r + 1) * M], ps[:],
                                         AF.Exp, bias=xdc_k[:, r:r + 1])
                    if r % 2 == 1:
                        rr = r // 2
                        nc.tensor.matmul(
                            ks[:], _pair(ones_pair8[:]),
                            _pair(kp8[:, rr * 1024:(rr + 1) * 1024]),
                            start=(rr == 0), stop=(rr == RC // 2 - 1),
                            perf_mode=DR)
                ks_st = small.tile([1, M], FP8, tag="ks_st")
                nc.scalar.activation(ks_st[:], ks[0:1, :], AF.Copy,
                                     scale=KS_SCALE)
                nc.sync.dma_start(cc_in[0:1, 0:M], ks_st[:])

            # ---- kptv'' d-major (v-stationary, DoubleRow), 8 banks ----
            with tc.tile_pool(name="psum_kptv", bufs=1,
                              space=bass.MemorySpace.PSUM) as psum_kptv:
                pk = {dt: psum_kptv.tile([128, M], F32,
                                         tag=f"pk{dt}", name=f"pk{dt}")
                      for dt in range(DT)}
                for rr in range(RC // 2):
                    for dt in pk:
                        nc.tensor.matmul(
                            pk[dt][:],
                            _pair(vt8[:, rr * 2048 + dt * 256:
                                      rr * 2048 + (dt + 1) * 256]),
                            _pair(kp8[:, rr * 1024:(rr + 1) * 1024]),
                            start=(rr == 0), stop=(rr == RC // 2 - 1),
                            perf_mode=DR)
                for dt in pk:
                    j, o = divmod(dt, 2)
                    for mt in range(MT):
                        nc.scalar.activation(
                            kv8[:, j * 1024 + mt * 256 + o * 128:
                                j * 1024 + mt * 256 + (o + 1) * 128],
                            pk[dt][:, mt * 128:(mt + 1) * 128],
                            AF.Copy, scale=KV_SCALE)

            # ---- C'' partial = kptv''^T @ proj_w^T  [m, dout] ----
            with tc.tile_pool(name="psum_C", bufs=2,
                              space=bass.MemorySpace.PSUM) as psum_C:
                for mt in range(MT):
                    jq, oq = divmod(mt, 2)
                    pc = psum_C.tile([128, D_MODEL], F32, tag="pc")
                    for j in range(4):
                        lhs = _pair(kv8[:, j * 1024 + mt * 256:
                                        j * 1024 + (mt + 1) * 256])
                        for h in range(2):
                            nc.tensor.matmul(
                                pc[:, h * 512:(h + 1) * 512], lhs,
                                _pair(pw8[:, j * 2048 + h * 1024:
                                          j * 2048 + (h + 1) * 1024]),
                                start=(j == 0), stop=(j == 3), perf_mode=DR)
                    st = outp.tile([128, D_MODEL], FP8, tag="ccst",
                                   name="ccst", bufs=2)
                    nc.scalar.activation(st[:], pc[:], AF.Copy)
                    for h in range(2):
                        nc.sync.dma_start(
                            cc_in[:, CC_COFF + jq * 2048 + h * 1024 + oq * 512:
                                  CC_COFF + jq * 2048 + h * 1024 + (oq + 1) * 512],
                            st[:, h * 512:(h + 1) * 512])

            # ---- pairwise AllReduce of C'' + ksum (fp8 payload) ----
            nc.gpsimd.collective_compute(
                "AllReduce", mybir.AluOpType.add, replica_groups=CC_GROUPS,
                ins=[cc_in.opt()], outs=[cc_out.opt()])
            nc.sync.dma_start(
                ksum8[:],
                cc_out[0:1, 0:M].rearrange("a (mt p) -> p (mt a)", p=128))
            nc.sync.dma_start(C8[:], cc_out[:, CC_COFF:CC_COFF + MT * D_MODEL])

            # ================= Q side (hides the AllReduce) ============
            with (
                tc.tile_pool(name="psum_wtxq", bufs=4,
                             space=bass.MemorySpace.PSUM) as psum_wtx,
                tc.tile_pool(name="psum_xdq", bufs=2,
                             space=bass.MemorySpace.PSUM) as psum_xd,
            ):
                for c in range(NCH):
                    lvl = []
                    for j in range(4):
                        for o in range(2):
                            sq = sqstream.tile([128, 512], BF16, tag="qsq",
                                               name=f"qsq{j}{o}", bufs=8)
                            sl = qt8[:, j * 4096 + c * 1024 + o * 512:
                                     j * 4096 + c * 1024 + (o + 1) * 512]
                            nc.vector.tensor_mul(sq[:], sl, sl)
                            lvl.append(sq)
                    depth = 0
                    while len(lvl) > 1:
                        nxt = []
                        for i in range(0, len(lvl), 2):
                            s = sqstream.tile([128, 512], BF16,
                                              tag=f"qsa{depth}",
                                              name=f"qsa{depth}_{i}",
                                              bufs=max(2, 4 >> depth))
                            nc.vector.tensor_add(s[:], lvl[i][:], lvl[i + 1][:])
                            nxt.append(s)
                        lvl = nxt
                        depth += 1
                    xdp = psum_xd.tile([1, 512], F32, tag="xdq")
                    nc.tensor.matmul(xdp[:], neghalf_col[:], lvl[0][:],
                                     start=True, stop=True)
                    nc.scalar.activation(xdT_q[0:1, c * 512:(c + 1) * 512],
                                         xdp[:], AF.Copy, bias=LOG64)
                for mt in range(MT):
                    jq, oq = divmod(mt, 2)
                    for c in range(NCH):
                        wqp = psum_wtx.tile([128, 512], F32, tag="wq")
                        for j in range(4):
                            nc.tensor.matmul(
                                wqp[:],
                                _pair(wq8[:, j * 1024 + mt * 256:
                                          j * 1024 + (mt + 1) * 256]),
                                _pair(qt8[:, j * 4096 + c * 1024:
                                          j * 4096 + (c + 1) * 1024]),
                                start=(j == 0), stop=False, perf_mode=DR)
                        nc.tensor.matmul(wqp[:], ones_row[:],
                                         xdT_q[0:1, c * 512:(c + 1) * 512],
                                         start=False, stop=True)
                        for rl in range(4):
                            r = c * 4 + rl
                            nc.scalar.activation(
                                qp8[:, jq * 4096 + r * 256 + oq * 128:
                                    jq * 4096 + r * 256 + (oq + 1) * 128],
                                wqp[:, rl * 128:(rl + 1) * 128], AF.Exp)

            # ---- HAM warm-keeper: paced dummy matmuls (CC insurance) ----
            with tc.tile_pool(name="psum_dummy", bufs=2,
                              space=bass.MemorySpace.PSUM) as psum_dummy:
                for i in range(N_DUMMY):
                    src, dst = (junkA, junkB) if i % 2 == 0 else (junkB, junkA)
                    nc.vector.tensor_copy(dst[:], src[:])
                    dp = psum_dummy.tile([128, 16], F32, tag="dp")
                    nc.tensor.matmul(dp[:], ident[:, 0:128],
                                     dst[:, 0:16], start=True, stop=True)

            # ---- OUT: out = po / (64*pD + 4096*m*eps) ----
            with (
                tc.tile_pool(name="psum_o", bufs=3,
                             space=bass.MemorySpace.PSUM) as psum_o,
                tc.tile_pool(name="psum_D", bufs=2,
                             space=bass.MemorySpace.PSUM) as psum_D,
            ):
                for r in range(RC):
                    po = psum_o.tile([128, D_MODEL], F32, tag="po")
                    pD = psum_D.tile([128, 1], F32, tag="pD")
                    for j in range(2):
                        lhs = _pair(qp8[:, j * 4096 + r * 256:
                                        j * 4096 + (r + 1) * 256])
                        for h in range(2):
                            nc.tensor.matmul(
                                po[:, h * 512:(h + 1) * 512], lhs,
                                _pair(C8[:, j * 2048 + h * 1024:
                                         j * 2048 + (h + 1) * 1024]),
                                start=(j == 0), stop=(j == 1), perf_mode=DR)
                    for mt in range(MT):
                        jq, oq = divmod(mt, 2)
                        nc.tensor.matmul(
                            pD[:],
                            qp8[:, jq * 4096 + r * 256 + oq * 128:
                                jq * 4096 + r * 256 + (oq + 1) * 128],
                            ksum8[:, mt:mt + 1],
                            start=(mt == 0), stop=(mt == MT - 1))
                    Dp = small.tile([128, 1], F32, tag="Dp")
                    recD = small.tile([128, 1], F32, tag="recD")
                    nc.scalar.activation(Dp[:], pD[:], AF.Copy,
                                         scale=DIV_SCALE, bias=DIV_BIAS)
                    nc.vector.reciprocal(recD[:], Dp[:])
                    ot = outp.tile([128, D_MODEL], BF16, tag="ot")
                    for h in range(2):
                        nc.vector.tensor_scalar_mul(
                            ot[:, h * 512:(h + 1) * 512],
                            po[:, h * 512:(h + 1) * 512], recD[:])
                    nc.sync.dma_start(out_d[r * 128:(r + 1) * 128, :], ot[:])

    nc.compile()
    return nc


_NC_CACHE = None


def _get_program():
    global _NC_CACHE
    if _NC_CACHE is None:
        _NC_CACHE = _build_program()
    return _NC_CACHE


def _c(a):
    return np.ascontiguousarray(a)


def _make_in_maps(q, k, v, w, proj_w):
    wT = w.T.astype(FP8_NP)          # [1024, 512]
    pwT = proj_w.T.astype(FP8_NP)    # [1024, 1024]
    wk = _c(wT.reshape(4, 2, 128, 512).transpose(2, 0, 1, 3)
            .reshape(128, 4096))
    wq = _c(wT.reshape(4, 2, 128, 4, 128).transpose(2, 0, 3, 1, 4)
            .reshape(128, 4096))
    pw = _c(pwT.reshape(4, 2, 128, 2, 512).transpose(2, 0, 3, 1, 4)
            .reshape(128, 8192))
    in_maps = []
    for c in range(N_CORES):
        b, h = divmod(c, 2)
        sl = slice(h * TC, (h + 1) * TC)
        kT = k[b, sl].T.astype(FP8_NP)   # [1024, 2048]
        qT = q[b, sl].T.astype(FP8_NP)
        vv = v[b, sl].astype(FP8_NP)     # [2048, 1024]
        in_maps.append({
            "k8": _c(kT.reshape(4, 2, 128, 16, 128).transpose(2, 3, 0, 1, 4)
                     .reshape(128, 16384)),
            "q8": _c(qT.reshape(4, 2, 128, 4, 512).transpose(2, 0, 3, 1, 4)
                     .reshape(128, 16384)),
            "v8": _c(vv.reshape(8, 2, 128, 8, 128).transpose(2, 0, 3, 1, 4)
                     .reshape(128, 16384)),
            "wk8": wk,
            "ident": np.eye(128, dtype=BF16_NP),
            "wq8": wq,
            "pw8": pw,
        })
    return in_maps


def run(q, k, v, w, proj_w, trace=False, tmpdir=None):
    nc = _get_program()
    in_maps = _make_in_maps(q, k, v, w, proj_w)
    res = bass_utils.run_bass_kernel_spmd(
        nc, in_maps, core_ids=list(range(N_CORES)), trace=trace,
        tmpdir=tmpdir)
    out = np.empty((B, T, D_MODEL), dtype=np.float32)
    for c in range(N_CORES):
        b, h = divmod(c, 2)
        out[b, h * TC:(h + 1) * TC] = res.results[c]["out"].astype(np.float32)
    return out, res


def kernel(q, k, v, w, proj_w):
    out, _ = run(np.asarray(q, dtype=np.float32),
                 np.asarray(k, dtype=np.float32),
                 np.asarray(v, dtype=np.float32),
                 np.asarray(w, dtype=np.float32),
                 np.asarray(proj_w, dtype=np.float32))
    return out
